# revision 29
# baseline (speedup 1.0000x reference)
"""Trainium2 Bass kernel for BatchChannelDecorrelationLoss.

Contract: kernel(**inputs) takes FULL unsharded inputs
  y:             (16, 192, 32, 32) f32
  x_hat:         (16, 3, 512, 512) f32
  target:        (16, 3, 512, 512) f32
  likelihoods_y: (16, 192, 32, 32) f32
and returns the FULL output: scalar f32 loss.

Strategy (data-parallel over batch N across 8 cores, 2 samples/core):
  device, per core:
    - per-(n,c) max / min of y over H*W (f32, exact)   -> stats (384, 2)
    - row-Gram B = Z^T Z over all 384 (n,c) rows, bf16 -> b0/b1/b2 tiles
      (upper block-triangle; host extracts the two per-sample 192x192
       diagonal blocks; bf16 is fine: corr term is ~1e-6 of the loss)
    - row sums via ones-vector matmul                  -> rs (1, 384)
    - (x_hat-target)^2 partial sums per partition      -> macc (128, 7)
    - sum(log(lik)) partial per partition              -> lnacc (128, 1)
  host:
    - rates = sum_n (round(max) - round(min))  [round commutes with max/min]
    - stable argsort -> top-64 channel idx  (matches jnp.argsort tie-break)
    - cov = (G_k - S_k S_k^T / M) / (M-1) on the selected 64x64 block
    - loss = lmbda*255^2*mse + bpp + lmbda_corr*sum(offdiag(cov)^2)

Engine/DMA choreography (engine streams execute in order, so program
order is placement):
  - sync HWDGE queue: y (packed 2 rows/partition -> 8 KB descriptor
    lines), lik, then the x_hat chunks; scalar HWDGE queue: target as
    three 2 MB blocks split into 8 KB descriptors whose queue-ring
    waits all resolve before ACT's first compute.
  - MSE chunk sizes shrink at the end so the post-last-byte tail is
    tiny.
  - DVE stream: max/min reduces then the subtracts (chunk-arrival
    paced).  ACT stream: bf16 casts, transpose PSUM->SBUF copies, Ln,
    Gram copies, squares; the critical macc store issues right after
    the last square.  All other stores go last on the sync queue.
"""

import math
import sys

if "/opt/trn_rl_repo" not in sys.path:
    sys.path.insert(0, "/opt/trn_rl_repo")

import numpy as np

import concourse.bacc as bacc
import concourse.masks as masks
import concourse.mybir as mybir
import concourse.tile as tile
from concourse.bass_utils import run_bass_kernel_spmd

# ---- problem constants (hardcoded per spec) ----
N, C, HY, WY = 16, 192, 32, 32
NI, CI, HI, WI = 16, 3, 512, 512
TOP_K = 64
LMBDA = 0.01
LMBDA_CORR = 1e-4
N_CORES = 8
NS = N // N_CORES          # samples per core = 2
YROWS = NS * C             # 384
YCOLS = HY * WY            # 1024
MSE_COLS = NS * CI * HI * WI // 128   # 12288
LIK_COLS = NS * C * HY * WY // 128    # 3072
MSE_CHUNKS = [2048, 2048, 2048, 2048, 2048, 1536, 512]   # sums to 12288
TG_BLOCKS = [4096, 4096, 4096]        # tg loads in three 2MB blocks
TG_BLOCK_OFF = [0, 4096, 8192]
TG_OF_CHUNK = [0, 0, 1, 1, 2, 2, 2]   # chunk -> tg block
DVE_SQ = set()                        # (tensor_tensor_reduce crashes this HW path)
N_MSE = len(MSE_CHUNKS)
NJ = YCOLS // 128                     # 8 hw chunks

FP32 = mybir.dt.float32
BF16 = mybir.dt.bfloat16
AX = mybir.AxisListType
OP = mybir.AluOpType
AF = mybir.ActivationFunctionType

_prog_cache = {}


def _build_program():
    nc = bacc.Bacc("TRN2", target_bir_lowering=False, debug=False,
                   num_devices=N_CORES)

    ys = nc.dram_tensor("ys", [YROWS // 2, 2 * YCOLS], FP32, kind="ExternalInput")
    xh = nc.dram_tensor("xh", [128, MSE_COLS], FP32, kind="ExternalInput")
    tg = nc.dram_tensor("tg", [128, MSE_COLS], FP32, kind="ExternalInput")
    lk = nc.dram_tensor("lk", [128, LIK_COLS], FP32, kind="ExternalInput")

    stats = nc.dram_tensor("stats", [YROWS // 2, 4], FP32, kind="ExternalOutput")
    b0 = nc.dram_tensor("b0", [128, YROWS], FP32, kind="ExternalOutput")
    b1 = nc.dram_tensor("b1", [128, YROWS], FP32, kind="ExternalOutput")
    b2 = nc.dram_tensor("b2", [128, YROWS], FP32, kind="ExternalOutput")
    rs = nc.dram_tensor("rs", [1, YROWS], FP32, kind="ExternalOutput")
    maccd = nc.dram_tensor("macc", [128, N_MSE], FP32, kind="ExternalOutput")
    lnd = nc.dram_tensor("lnacc", [128, 1], FP32, kind="ExternalOutput")

    chunk_off = [0]
    for w in MSE_CHUNKS:
        chunk_off.append(chunk_off[-1] + w)

    with tile.TileContext(nc) as tc:
        with (
            tc.tile_pool(name="singles", bufs=1) as singles,
            tc.tile_pool(name="ypool", bufs=3) as ypool,
            tc.tile_pool(name="ybf", bufs=3) as ybfp,
            tc.tile_pool(name="ztp", bufs=8) as ztp,
            tc.tile_pool(name="stp", bufs=3) as stp,
            tc.tile_pool(name="mx", bufs=1) as mxp,
            tc.tile_pool(name="mt", bufs=1) as mtp,
            tc.tile_pool(name="lkp", bufs=1) as lkp,
            tc.tile_pool(name="sqs", bufs=2) as sqscr,
            tc.tile_pool(name="tpsum", bufs=4, space="PSUM") as tpsum,
            tc.tile_pool(name="gpsum", bufs=1, space="PSUM") as gpsum,
        ):
            # ---- loads ----
            # scalar queue: only early items (its ring waits resolve
            # before ACT compute); sync queue: everything else.
            # sync queue: y (packed, 8KB lines), lik, xh chunks.
            # scalar queue: tg as three 2MB blocks split into 8KB
            # descriptors (fair round-robin vs sync; ring waits resolve
            # before ACT's first compute).
            yA = ypool.tile([128, 2 * YCOLS], FP32, tag="yA")
            nc.sync.dma_start(yA[:], ys[0:128, :])
            yB = ypool.tile([64, 2 * YCOLS], FP32, tag="yB")
            nc.sync.dma_start(yB[:], ys[128:192, :])

            lt = lkp.tile([128, LIK_COLS], FP32)
            nc.sync.dma_start(lt[:], lk[:])

            mse_x = [mxp.tile([128, w], FP32, tag=f"xt{i}", name=f"xt{i}")
                     for i, w in enumerate(MSE_CHUNKS)]
            for i in range(N_MSE):
                nc.sync.dma_start(mse_x[i][:],
                                  xh[:, chunk_off[i]:chunk_off[i + 1]])

            tg_b = [mtp.tile([128, TG_BLOCKS[b]], FP32, tag=f"tb{b}",
                             name=f"tb{b}") for b in range(3)]
            for b in range(3):
                o = TG_BLOCK_OFF[b]
                nc.scalar.dma_start(tg_b[b][:], tg[:, o:o + TG_BLOCKS[b]],
                                    max_dma_last_dim=2048)

            ident = singles.tile([128, 128], BF16)
            masks.make_identity(nc, ident[:])
            ones = singles.tile([128, 1], BF16)
            nc.gpsimd.memset(ones[:], 1.0)
            macc = singles.tile([128, N_MSE], FP32)
            lnacc = singles.tile([128, 1], FP32)

            # ---- ACT: bf16 casts first (feed the PE chain) ----
            yAb = ybfp.tile([128, 2 * YCOLS], BF16, tag="yAb")
            nc.scalar.copy(yAb[:], yA[:])
            yBb = ybfp.tile([64, 2 * YCOLS], BF16, tag="yBb")
            nc.scalar.copy(yBb[:], yB[:])

            # ---- PE transposes into one PSUM tile per hw-chunk; one
            # DVE copy moves all 384 columns to SBUF.  Column k of zt
            # holds y-row perm[k] (see host-side PERM).
            zts = []
            for j in range(NJ):
                sl = slice(j * 128, (j + 1) * 128)
                sl2 = slice(YCOLS + j * 128, YCOLS + (j + 1) * 128)
                zt = ztp.tile([128, YROWS], BF16, tag="zt")
                pt = tpsum.tile([128, YROWS], BF16, tag="tp")
                nc.tensor.transpose(pt[:, 0:128], yAb[:, sl], ident[:])
                nc.tensor.transpose(pt[:, 128:256], yAb[:, sl2], ident[:])
                nc.tensor.transpose(pt[:, 256:320], yBb[:, sl],
                                    ident[0:64, 0:64])
                nc.tensor.transpose(pt[:, 320:384], yBb[:, sl2],
                                    ident[0:64, 0:64])
                nc.scalar.copy(zt[:], pt[:])
                zts.append(zt)

            nc.scalar.activation(lt[:], lt[:], AF.Ln,
                                 accum_out=lnacc[:, 0:1])

            # ---- DVE: per-row max/min on the packed views ----
            stA = stp.tile([128, 4], FP32, tag="stA")
            yA3 = yA[:].rearrange("p (two c) -> p two c", two=2)
            nc.vector.tensor_reduce(stA[:, 0:2], yA3, axis=AX.X, op=OP.max)
            nc.vector.tensor_reduce(stA[:, 2:4], yA3, axis=AX.X, op=OP.min)
            stB = stp.tile([64, 4], FP32, tag="stB")
            yB3 = yB[:].rearrange("p (two c) -> p two c", two=2)
            nc.vector.tensor_reduce(stB[:, 0:2], yB3, axis=AX.X, op=OP.max)
            nc.vector.tensor_reduce(stB[:, 2:4], yB3, axis=AX.X, op=OP.min)

            def mse_chunk(i):
                xt = mse_x[i]
                b = TG_OF_CHUNK[i]
                lo = chunk_off[i] - TG_BLOCK_OFF[b]
                tt = tg_b[b][:, lo:lo + MSE_CHUNKS[i]]
                nc.vector.tensor_tensor(xt[:], xt[:], tt, op=OP.subtract)
                if i in DVE_SQ:
                    # tail chunks: square+accumulate on DVE so the ACT
                    # square chain doesn't serialize the kernel tail
                    sq = sqscr.tile([128, MSE_CHUNKS[i]], FP32, tag="sqs")
                    nc.vector.tensor_tensor_reduce(
                        out=sq[:], in0=xt[:], in1=xt[:], scale=1.0,
                        scalar=0.0, op0=OP.mult, op1=OP.add,
                        accum_out=macc[:, i:i + 1])
                else:
                    nc.scalar.activation(xt[:], xt[:], AF.Square,
                                         accum_out=macc[:, i:i + 1])

            mse_chunk(0)
            mse_chunk(1)

            # ---- row-Gram upper blocks + row sums, PSUM-accumulated ----
            pb0 = gpsum.tile([128, YROWS], FP32, tag="pb0")
            for j, zt in enumerate(zts):
                nc.tensor.matmul(pb0[:], lhsT=zt[:, 0:128], rhs=zt[:],
                                 start=(j == 0), stop=(j == NJ - 1))
            pb1 = gpsum.tile([128, YROWS], FP32, tag="pb1")
            for j, zt in enumerate(zts):
                nc.tensor.matmul(pb1[:], lhsT=zt[:, 128:256], rhs=zt[:],
                                 start=(j == 0), stop=(j == NJ - 1))
            pb2 = gpsum.tile([128, YROWS], FP32, tag="pb2")
            for j, zt in enumerate(zts):
                nc.tensor.matmul(pb2[:], lhsT=zt[:, 256:384], rhs=zt[:],
                                 start=(j == 0), stop=(j == NJ - 1))
            prs = gpsum.tile([1, YROWS], FP32, tag="prs")
            for j, zt in enumerate(zts):
                nc.tensor.matmul(prs[:], lhsT=ones[:], rhs=zt[:],
                                 start=(j == 0), stop=(j == NJ - 1))

            mse_chunk(2)

            # ---- DVE: Gram PSUM -> SBUF while chunk 3 streams in ----
            gsb = []
            for psum_t, dram_t, w in ((pb0, b0, YROWS), (pb1, b1, YROWS),
                                      (pb2, b2, YROWS)):
                sb = singles.tile([128, w], FP32, tag=f"sb_{dram_t.name}",
                                  name=f"gout_{dram_t.name}")
                nc.scalar.copy(sb[:], psum_t[:])
                gsb.append((sb, dram_t))
            rssb = singles.tile([1, YROWS], FP32)
            nc.scalar.copy(rssb[:], prs[:])

            for i in range(3, N_MSE):
                mse_chunk(i)

            # critical-path store: right after the last square on ACT
            nc.scalar.dma_start(maccd[:], macc[:])

            # non-critical stores at the very end on the sync queue
            nc.sync.dma_start(stats[0:128, :], stA[:])
            nc.sync.dma_start(stats[128:192, :], stB[:])
            for sb, dram_t in gsb:
                nc.sync.dma_start(dram_t[:], sb[:])
            nc.sync.dma_start(rs[:], rssb[:])
            nc.sync.dma_start(lnd[:], lnacc[:])

    nc.compile()
    return nc


def _get_program():
    if "nc" not in _prog_cache:
        _prog_cache["nc"] = _build_program()
    return _prog_cache["nc"]


def kernel(y, x_hat, target, likelihoods_y):
    y = np.ascontiguousarray(y, dtype=np.float32)
    x_hat = np.ascontiguousarray(x_hat, dtype=np.float32)
    target = np.ascontiguousarray(target, dtype=np.float32)
    lik = np.ascontiguousarray(likelihoods_y, dtype=np.float32)

    nc = _get_program()

    in_maps = []
    for c in range(N_CORES):
        s = slice(c * NS, (c + 1) * NS)
        in_maps.append({
            "ys": y[s].reshape(YROWS // 2, 2 * YCOLS),
            "xh": x_hat[s].reshape(128, MSE_COLS),
            "tg": target[s].reshape(128, MSE_COLS),
            "lk": lik[s].reshape(128, LIK_COLS),
        })

    res = run_bass_kernel_spmd(nc, in_maps, list(range(N_CORES)))
    results = res.results

    # ---- host-side combine (all O(C^2) and smaller) ----
    # stats: partition p holds y-rows (2p, 2p+1) -- natural order
    stats = np.stack([r["stats"] for r in results])       # (8, 192, 4)
    fmax = stats[:, :, 0:2].reshape(N_CORES, YROWS).reshape(N, C)
    fmin = stats[:, :, 2:4].reshape(N_CORES, YROWS).reshape(N, C)

    # rates: round commutes with max/min; np.round == jnp.round (half-to-even)
    per_sample = np.round(fmax).astype(np.int64) - np.round(fmin).astype(np.int64)
    rates = per_sample.sum(axis=0)                        # (192,)
    idx = np.argsort(rates, kind="stable")[::-1][:TOP_K]

    # row-Gram: zt column k holds y-row PERM[k]; B[PERM[i],PERM[j]] = B'[i,j]
    perm = np.concatenate([np.arange(0, 256, 2), np.arange(1, 256, 2),
                           np.arange(256, 384, 2), np.arange(257, 384, 2)])
    Bp = np.zeros((YROWS, YROWS), dtype=np.float64)
    for r in results:
        Bp[0:128, :] += r["b0"]
        Bp[128:256, :] += r["b1"]
        Bp[256:384, :] += r["b2"]
    B = np.zeros((YROWS, YROWS), dtype=np.float64)
    B[np.ix_(perm, perm)] = Bp
    G = B[0:C, 0:C] + B[C:2 * C, C:2 * C]

    rs_all = np.sum([r["rs"] for r in results], axis=0,
                    dtype=np.float64).reshape(YROWS)
    S = np.zeros(YROWS)
    S[perm] = rs_all
    S = S[0:C] + S[C:2 * C]

    M = N * HY * WY                                       # 16384
    Gk = G[np.ix_(idx, idx)]
    Sk = S[idx]
    cov = (Gk - np.outer(Sk, Sk) / M) / (M - 1)
    off = cov - np.diag(np.diag(cov))
    corr_loss = float(np.sum(off ** 2))

    mse_sum = float(np.sum([r["macc"] for r in results], dtype=np.float64))
    ln_sum = float(np.sum([r["lnacc"] for r in results], dtype=np.float64))

    num_pixels = N * HI * WI
    mse_loss = mse_sum / (NI * CI * HI * WI)
    bpp_loss = ln_sum / (-math.log(2) * num_pixels)
    loss = LMBDA * 255.0 ** 2 * mse_loss + bpp_loss + LMBDA_CORR * corr_loss
    return np.asarray(loss, dtype=np.float32)
